# revision 8
# baseline (speedup 1.0000x reference)
"""Distributed GQA attention kernel for 8 TRN2 NeuronCores.

Strategy (tensor-parallel over heads, A2A re-shard before o_proj):
  - Core i owns q heads 4i..4i+3 and kv head i (GQA group) -> attention is
    fully local per core.
  - QKV projection computed TRANSPOSED (features on partitions):
      qkvT[f, s] = w_qkv_shard @ hidden.T   (lhsT = w_qkv_shard.T, rhs = hidden.T)
  - RoPE (neox) applied in [d, s] layout via elementwise DVE ops.
  - Attention in "scoresT" layout: scoresT[k, q] = kT.T @ qT (K=d=64 contraction,
    row-packed 2 heads at base partitions 0/64). Softmax without max-subtraction
    (scores are O(1)); exp on ACT; causal masking via 0/1 mask multiply after exp
    on diagonal blocks; PV matmul with ones-augmented v gives both ctxT and the
    softmax denominators in one accumulation.
  - AllToAll re-shards ctxT from head-sharded to sequence-sharded (1 MB bf16
    per rank instead of a 16 MB f32 AllReduce).
  - o_proj locally on own 256 seq rows with full w_o.T -> output row-sharded,
    host concatenates.
"""

import os
import numpy as np
import ml_dtypes

import concourse.bass as bass
import concourse.mybir as mybir
from concourse import bacc, tile

F32 = mybir.dt.float32
BF16 = mybir.dt.bfloat16
BF16_NP = ml_dtypes.bfloat16

# Problem constants (hardcoded per harness contract)
H = 2048
S = 2048
NH = 32
NKV = 8
HD = 64
Q_SIZE = NH * HD      # 2048
KV_SIZE = NKV * HD    # 512
NCORES = 8
QH = NH // NCORES     # 4 q heads per core
ROPE_THETA = 10000.0

P = 128
KT = H // P           # 16 contraction tiles over H
NQ = 512              # query chunk (matmul moving free dim)
NJC = S // NQ         # 4 query chunks
NKB = S // P          # 16 key tiles
SROWS = S // NCORES   # 256 seq rows per core after A2A

_NC_CACHE = None
LAST_RESULTS = None


def _build_nc():
    nc = bacc.Bacc(
        "TRN2",
        target_bir_lowering=False,
        debug=False,
        num_devices=NCORES,
    )

    # ---- I/O ----
    hT_d = nc.dram_tensor("hT", [P, KT * S], BF16, kind="ExternalInput")
    wq_d = nc.dram_tensor("wqkvT", [P, KT * 384], BF16, kind="ExternalInput")
    wo_d = nc.dram_tensor("woT", [P, KT * H], BF16, kind="ExternalInput")
    ropec_d = nc.dram_tensor("ropec", [P, S], F32, kind="ExternalInput")
    ropes_d = nc.dram_tensor("ropes", [P, S], F32, kind="ExternalInput")
    masks_d = nc.dram_tensor("masks", [P, 4 * NQ], BF16, kind="ExternalInput")
    ident_d = nc.dram_tensor("ident", [P, P], F32, kind="ExternalInput")
    out_d = nc.dram_tensor("out", [SROWS, H], F32, kind="ExternalOutput")

    rg = [list(range(NCORES))]

    with tile.TileContext(nc) as tc:
        with (
            tc.tile_pool(name="dram", bufs=1, space="DRAM") as dram,
            tc.tile_pool(name="const", bufs=1) as const,
            tc.tile_pool(name="qk", bufs=1) as qkpool,
            tc.tile_pool(name="ps_mm", bufs=2, space="PSUM") as ps_mm,
            tc.tile_pool(name="ps_s", bufs=2, space="PSUM") as ps_s,
            tc.tile_pool(name="ps_ctx", bufs=1, space="PSUM") as ps_ctx,
            tc.tile_pool(name="ps_bc", bufs=1, space="PSUM") as ps_bc,
            tc.tile_pool(name="esb", bufs=4) as esb,
            tc.tile_pool(name="small", bufs=2) as small,
            tc.tile_pool(name="outp", bufs=2) as outp,
        ):
            cc_in = dram.tile([S, SROWS], BF16, tag="cc_in")
            cc_out = dram.tile([S, SROWS], BF16, tag="cc_out")

            # ---- persistent SBUF ----
            ropec = const.tile([P, S], F32, tag="ropec")
            nc.sync.dma_start(ropec[:], ropec_d[:])
            ropes = const.tile([P, S], F32, tag="ropes")
            nc.sync.dma_start(ropes[:], ropes_d[:])
            masks = const.tile([P, 4 * NQ], BF16, tag="masks")
            nc.sync.dma_start(masks[:], masks_d[:])
            ident = const.tile([P, P], F32, tag="ident")
            nc.sync.dma_start(ident[:], ident_d[:])
            ones64 = const.tile([1, 64], F32, tag="ones64")
            nc.vector.memset(ones64[:], 1.0)

            qpair = [qkpool.tile([P, S], BF16, tag=f"qpair{m}", name=f"qpair{m}") for m in range(2)]
            kdup = qkpool.tile([P, S], BF16, tag="kdup")
            v_aug = qkpool.tile([P, NKB * 65], BF16, tag="v_aug")
            cc_sb = qkpool.tile([P, KT * SROWS], BF16, tag="cc_sb")

            # ========== Phase 1: QKV projection (transposed) ==========
            with (
                tc.tile_pool(name="ph1", bufs=1) as ph1,
                tc.tile_pool(name="qkvf", bufs=1) as qkvf,
                tc.tile_pool(name="ropetmp", bufs=1) as ropetmp,
            ):
                h_sb = ph1.tile([P, KT * S], BF16, tag="h_sb")
                wq_sb = ph1.tile([P, KT * 384], BF16, tag="wq_sb")
                for kt in range(KT):
                    nc.sync.dma_start(
                        h_sb[:, kt * S : (kt + 1) * S], hT_d[:, kt * S : (kt + 1) * S]
                    )
                    nc.sync.dma_start(
                        wq_sb[:, kt * 384 : (kt + 1) * 384],
                        wq_d[:, kt * 384 : (kt + 1) * 384],
                    )

                qkv_sb = qkvf.tile([P, 3 * S], F32, tag="qkv_sb")

                for m in range(3):
                    for n in range(NJC):
                        ps = ps_mm.tile([P, NQ], F32, tag="mm")
                        for kt in range(KT):
                            nc.tensor.matmul(
                                ps[:],
                                wq_sb[:, kt * 384 + 128 * m : kt * 384 + 128 * (m + 1)],
                                h_sb[:, kt * S + NQ * n : kt * S + NQ * (n + 1)],
                                start=(kt == 0),
                                stop=(kt == KT - 1),
                            )
                        nc.vector.tensor_copy(
                            qkv_sb[:, m * S + NQ * n : m * S + NQ * (n + 1)], ps[:]
                        )

                # ========== Phase 2: RoPE ==========
                # q head pairs (m = 0, 1), full 128 partitions (2 heads each)
                for m in range(2):
                    x = qkv_sb[:, m * S : (m + 1) * S]
                    swp = ropetmp.tile([P, S], F32, tag="swp")
                    for b in range(2):  # two 64-row head blocks
                        o = 64 * b
                        nc.vector.tensor_copy(
                            swp[o : o + 32, :], qkv_sb[o + 32 : o + 64, m * S : (m + 1) * S]
                        )
                        nc.vector.tensor_copy(
                            swp[o + 32 : o + 64, :], qkv_sb[o : o + 32, m * S : (m + 1) * S]
                        )
                    prod = ropetmp.tile([P, S], F32, tag="prod")
                    nc.vector.tensor_mul(prod[:], x, ropec[:])
                    prod2 = ropetmp.tile([P, S], F32, tag="prod2")
                    nc.vector.tensor_mul(prod2[:], swp[:], ropes[:])
                    nc.vector.tensor_add(qpair[m][:], prod[:], prod2[:])

                # k head (qkv_sb m=2 block, partitions 0..63)
                xk = qkv_sb[0:64, 2 * S : 3 * S]
                swpk = ropetmp.tile([P, S], F32, tag="swp")
                nc.vector.tensor_copy(swpk[0:32, :], qkv_sb[32:64, 2 * S : 3 * S])
                nc.vector.tensor_copy(swpk[32:64, :], qkv_sb[0:32, 2 * S : 3 * S])
                prodk = ropetmp.tile([P, S], F32, tag="prod")
                nc.vector.tensor_mul(prodk[0:64, :], xk, ropec[0:64, :])
                prodk2 = ropetmp.tile([P, S], F32, tag="prod2")
                nc.vector.tensor_mul(prodk2[0:64, :], swpk[0:64, :], ropes[0:64, :])
                nc.vector.tensor_add(kdup[0:64, :], prodk[0:64, :], prodk2[0:64, :])
                nc.vector.tensor_add(kdup[64:128, :], prodk[0:64, :], prodk2[0:64, :])

                # ========== Phase 3: transpose v -> v_aug [keys, 64+1] ==========
                for kb in range(NKB):
                    vps = ps_mm.tile([P, 64], F32, tag="mm")
                    nc.tensor.transpose(
                        vps[:, 0:64],
                        qkv_sb[64:128, 2 * S + P * kb : 2 * S + P * (kb + 1)],
                        ident[64:128, 64:128],
                    )
                    nc.vector.tensor_copy(v_aug[:, kb * 65 : kb * 65 + 64], vps[:, 0:64])
                    nc.vector.memset(v_aug[:, kb * 65 + 64 : kb * 65 + 65], 1.0)

            # ========== Phase 4: attention per head pair ==========
            for p in range(2):
                for jc in range(NJC):
                    nkb = 4 * (jc + 1)
                    ctxs = [ps_ctx.tile([P, NQ], F32, tag=f"ctx{hh}", name=f"ctx{hh}") for hh in range(2)]
                    for kb in range(nkb):
                        for hh in range(2):
                            base = 64 * hh
                            sp = ps_s.tile([P, NQ], F32, tag="sp")
                            nc.tensor.matmul(
                                sp[:],
                                kdup[base : base + 64, P * kb : P * (kb + 1)],
                                qpair[p][base : base + 64, NQ * jc : NQ * (jc + 1)],
                                start=True,
                                stop=True,
                            )
                            e = esb.tile([P, NQ], BF16, tag="e")
                            nc.scalar.activation(
                                e[:], sp[:], mybir.ActivationFunctionType.Exp,
                                scale=0.125,
                            )
                            d = kb - 4 * jc
                            if d >= 0:
                                nc.vector.tensor_mul(
                                    e[:], e[:], masks[:, NQ * d : NQ * (d + 1)]
                                )
                            nc.tensor.matmul(
                                ctxs[hh][0:65, :],
                                v_aug[:, kb * 65 : kb * 65 + 65],
                                e[:],
                                start=(kb == 0),
                                stop=(kb == nkb - 1),
                            )
                    for hh in range(2):
                        h = 2 * p + hh
                        rec = small.tile([1, NQ], F32, tag="rec")
                        nc.vector.reciprocal(rec[:], ctxs[hh][64:65, :])
                        bc = ps_bc.tile([64, NQ], F32, tag="bc")
                        nc.tensor.matmul(
                            bc[:], ones64[:], rec[:], start=True, stop=True
                        )
                        bcs = small.tile([64, NQ], F32, tag="bcs")
                        nc.vector.tensor_copy(bcs[:], bc[:])
                        ctxn = small.tile([64, NQ], BF16, tag="ctxn")
                        nc.vector.tensor_mul(ctxn[:], ctxs[hh][0:64, :], bcs[:])
                        # scatter into A2A input: shard j holds my ctxT cols 256j..
                        for half in range(2):
                            j = 2 * jc + half
                            nc.sync.dma_start(
                                cc_in[SROWS * j + 64 * h : SROWS * j + 64 * (h + 1), :],
                                ctxn[:, SROWS * half : SROWS * (half + 1)],
                            )

            # ========== Phase 5: AllToAll ==========
            nc.gpsimd.collective_compute(
                "AllToAll",
                mybir.AluOpType.bypass,
                replica_groups=rg,
                ins=[cc_in.opt()],
                outs=[cc_out.opt()],
            )

            # ========== Phase 6: o_proj on own seq rows ==========
            for kt in range(KT):
                nc.sync.dma_start(
                    cc_sb[:, kt * SROWS : (kt + 1) * SROWS],
                    cc_out[kt * P : (kt + 1) * P, :],
                )
            wo_ctx = tc.tile_pool(name="wo_stream", bufs=2)
            wo_stream = wo_ctx.__enter__()
            for n in range(NJC):
                wos = wo_stream.tile([P, KT * NQ], BF16, tag="wos")
                for kt in range(KT):
                    nc.sync.dma_start(
                        wos[:, kt * NQ : (kt + 1) * NQ],
                        wo_d[:, kt * H + NQ * n : kt * H + NQ * (n + 1)],
                    )
                for m in range(2):
                    ps = ps_mm.tile([P, NQ], F32, tag="mm")
                    for kt in range(KT):
                        nc.tensor.matmul(
                            ps[:],
                            cc_sb[:, kt * SROWS + P * m : kt * SROWS + P * (m + 1)],
                            wos[:, kt * NQ : (kt + 1) * NQ],
                            start=(kt == 0),
                            stop=(kt == KT - 1),
                        )
                    ot = outp.tile([P, NQ], F32, tag="ot")
                    nc.vector.tensor_copy(ot[:], ps[:])
                    nc.sync.dma_start(
                        out_d[P * m : P * (m + 1), NQ * n : NQ * (n + 1)], ot[:]
                    )
            wo_ctx.__exit__(None, None, None)

    nc.compile()
    return nc


def _get_nc():
    global _NC_CACHE
    if _NC_CACHE is None:
        _NC_CACHE = _build_nc()
    return _NC_CACHE


def _stage_inputs(position_ids, hidden_states, w_qkv, w_o):
    """Host-side sharding / layout staging. Returns in_maps for 8 cores."""
    pos = np.asarray(position_ids)[0].astype(np.float32)            # [S]
    hidden = np.asarray(hidden_states, dtype=np.float32)[0]         # [S, H]
    w_qkv = np.asarray(w_qkv, dtype=np.float32)                     # [3072, H]
    w_o = np.asarray(w_o, dtype=np.float32)                         # [H, Q_SIZE]

    # hT tiles: [H, S] -> [128, KT*S] (k-tile kt at cols kt*S..)
    hT = np.ascontiguousarray(hidden.T)
    hT_r = np.ascontiguousarray(
        hT.reshape(KT, P, S).transpose(1, 0, 2).reshape(P, KT * S)
    ).astype(BF16_NP)

    # w_o.T tiles: [Q_SIZE, H] -> [128, KT*H]
    woT = np.ascontiguousarray(w_o.T)
    woT_r = np.ascontiguousarray(
        woT.reshape(KT, P, H).transpose(1, 0, 2).reshape(P, KT * H)
    ).astype(BF16_NP)

    # rope tables in [d, s] layout for a [128 = 2 heads x 64] tile
    inv_freq = (1.0 / (ROPE_THETA ** (np.arange(0, HD, 2, dtype=np.float32) / HD)))
    ang = pos[:, None] * inv_freq[None, :]                          # [S, 32]
    cosT = np.cos(ang).T.astype(np.float32)                         # [32, S]
    sinT = np.sin(ang).T.astype(np.float32)
    ropec = np.concatenate([cosT, cosT, cosT, cosT], axis=0)        # [128, S]
    ropes = np.concatenate([-sinT, sinT, -sinT, sinT], axis=0)      # [128, S]

    # causal masks for the 4 diagonal block offsets: mask_d[p, f] = p + 128 d <= f
    f = np.arange(NQ)
    mask_list = []
    for d in range(4):
        pp = np.arange(P)[:, None] + 128 * d
        mask_list.append((pp <= f[None, :]).astype(BF16_NP))
    masks = np.concatenate(mask_list, axis=1)                       # [128, 4*NQ]

    ident = np.eye(P, dtype=np.float32)

    in_maps = []
    for i in range(NCORES):
        rows_q = w_qkv[QH * HD * i : QH * HD * (i + 1)]             # [256, H]
        row_k = w_qkv[Q_SIZE + HD * i : Q_SIZE + HD * (i + 1)]      # [64, H]
        row_v = w_qkv[Q_SIZE + KV_SIZE + HD * i : Q_SIZE + KV_SIZE + HD * (i + 1)]
        wshard = np.concatenate([rows_q, row_k, row_v], axis=0)     # [384, H]
        wqkvT = np.ascontiguousarray(wshard.T)                      # [H, 384]
        wqkvT_r = np.ascontiguousarray(
            wqkvT.reshape(KT, P, 384).transpose(1, 0, 2).reshape(P, KT * 384)
        ).astype(BF16_NP)
        in_maps.append(
            {
                "hT": hT_r,
                "wqkvT": wqkvT_r,
                "woT": woT_r,
                "ropec": ropec,
                "ropes": ropes,
                "masks": masks,
                "ident": ident,
            }
        )
    return in_maps


def _ensure_ntff_hook():
    """The container's antenv stub lacks axon_hooks, so trn_boot silently
    skipped NTFF hook registration. Recreate the module and register the
    ctypes-based hook so run_bass_kernel_spmd(trace=True) can profile."""
    import sys
    import types

    if "antenv.axon_hooks" in sys.modules:
        return
    try:
        import antenv
        from trn_agent_boot.trn_boot import _ntff_profile_via_ctypes

        hooks = types.ModuleType("antenv.axon_hooks")
        _state = {}

        def set_axon_ntff_profile_hook(h):
            _state["h"] = h

        def get_axon_ntff_profile_hook():
            return _state.get("h")

        hooks.set_axon_ntff_profile_hook = set_axon_ntff_profile_hook
        hooks.get_axon_ntff_profile_hook = get_axon_ntff_profile_hook
        sys.modules["antenv.axon_hooks"] = hooks
        antenv.axon_hooks = hooks
        hook = _ntff_profile_via_ctypes("/opt/axon/libaxon_pjrt.so")
        if hook is not None:
            set_axon_ntff_profile_hook(hook)
    except Exception:
        pass


def kernel(**inputs):
    global LAST_RESULTS
    from concourse.bass_utils import run_bass_kernel_spmd

    nc = _get_nc()
    in_maps = _stage_inputs(
        inputs["position_ids"], inputs["hidden_states"], inputs["w_qkv"], inputs["w_o"]
    )
    trace = os.environ.get("KERNEL_TRACE", "0") == "1"
    if trace:
        _ensure_ntff_hook()
    res = run_bass_kernel_spmd(
        nc, in_maps, core_ids=list(range(NCORES)), trace=trace
    )
    LAST_RESULTS = res
    outs = [np.asarray(res.results[i]["out"], dtype=np.float32) for i in range(NCORES)]
    full = np.concatenate(outs, axis=0)                             # [S, H]
    return full.reshape(1, S, H)


# revision 14
# speedup vs baseline: 1.2613x; 1.2613x over previous
"""Distributed GQA attention kernel for 8 TRN2 NeuronCores.

Strategy (tensor-parallel over heads, A2A re-shard before o_proj):
  - Core i owns q heads 4i..4i+3 and kv head i (GQA group) -> attention is
    fully local per core.
  - QKV projection computed TRANSPOSED (features on partitions):
      qkvT[f, s] = w_qkv_shard @ hidden.T   (lhsT = w_qkv_shard.T, rhs = hidden.T)
  - RoPE (neox) applied in [d, s] layout via elementwise DVE ops.
  - Attention in "scoresT" layout: scoresT[k, q] = kT.T @ qT (K=d=64 contraction,
    row-packed head pairs at base partitions 0/64 run concurrently on the PE).
    Softmax without max-subtraction (scores are O(1)); exp on ACT; causal
    masking via 0/1 mask multiply after exp on diagonal blocks; PV matmul with
    ones-augmented v gives both ctxT and the softmax denominators in one
    accumulation; denominators broadcast via gpsimd.partition_broadcast.
  - Two AllToAlls (one per head pair) re-shard ctxT from head-sharded to
    sequence-sharded; the first overlaps the second pair's attention; a tiny
    warm-up collective at kernel start absorbs the ncfw cold-start delay.
  - o_proj locally on own 256 seq rows with full w_o.T, split into an even-kt
    pass (only needs the first A2A) that hides under the second A2A, then an
    odd-kt pass. Output row-sharded, host concatenates.
"""

import os
import numpy as np
import ml_dtypes

import concourse.bass as bass
import concourse.mybir as mybir
from concourse import bacc, tile
import bass_rust as _br

F32 = mybir.dt.float32
BF16 = mybir.dt.bfloat16
BF16_NP = ml_dtypes.bfloat16

# Problem constants (hardcoded per harness contract)
H = 2048
S = 2048
NH = 32
NKV = 8
HD = 64
Q_SIZE = NH * HD      # 2048
KV_SIZE = NKV * HD    # 512
NCORES = 8
QH = NH // NCORES     # 4 q heads per core
ROPE_THETA = 10000.0

P = 128
KT = H // P           # 16 contraction tiles over H
NQ = 512              # query chunk (matmul moving free dim)
NJC = S // NQ         # 4 query chunks
NKB = S // P          # 16 key tiles
SROWS = S // NCORES   # 256 seq rows per core after A2A

_NC_CACHE = None
LAST_RESULTS = None


def _build_nc():
    nc = bacc.Bacc(
        "TRN2",
        target_bir_lowering=False,
        debug=False,
        num_devices=NCORES,
    )

    # ---- I/O ----
    hT_d = nc.dram_tensor("hT", [P, KT * S], BF16, kind="ExternalInput")
    wq_d = nc.dram_tensor("wqkvT", [P, KT * 384], BF16, kind="ExternalInput")
    wo_d = nc.dram_tensor("woT", [P, KT * H], BF16, kind="ExternalInput")
    ropec_d = nc.dram_tensor("ropec", [P, S], F32, kind="ExternalInput")
    ropes_d = nc.dram_tensor("ropes", [P, S], F32, kind="ExternalInput")
    masks_d = nc.dram_tensor("masks", [P, 4 * 2 * NQ], BF16, kind="ExternalInput")
    ident_d = nc.dram_tensor("ident", [P, P], F32, kind="ExternalInput")
    out_d = nc.dram_tensor("out", [SROWS, H], F32, kind="ExternalOutput")

    rg = [list(range(NCORES))]

    with tile.TileContext(nc) as tc:
        with (
            tc.tile_pool(name="dram", bufs=1, space="DRAM") as dram,
            tc.tile_pool(name="const", bufs=1) as const,
            tc.tile_pool(name="qk", bufs=1) as qkpool,
            tc.tile_pool(name="esb", bufs=4) as esb,
            tc.tile_pool(name="small", bufs=2) as small,
            tc.tile_pool(name="outp", bufs=2) as outp,
        ):
            # A2A buffers, one per head pair (shard j rows = my pair ctxT for
            # seq cols of core j)
            cc_in = [
                dram.tile([NCORES * P, SROWS], BF16, tag=f"cc_in{p}", name=f"cc_in{p}")
                for p in range(2)
            ]
            cc_out = [
                dram.tile([NCORES * P, SROWS], BF16, tag=f"cc_out{p}", name=f"cc_out{p}")
                for p in range(2)
            ]

            # tiny warm-up collective absorbs the ~11.5us ncfw first-collective
            # start delay so the real A2As get the warm path
            ccw_in = dram.tile([NCORES, 16], BF16, tag="ccw_in")
            ccw_out = dram.tile([NCORES, 16], BF16, tag="ccw_out")
            warm_sb = const.tile([NCORES, 16], BF16, tag="warm_sb")
            nc.vector.memset(warm_sb[:], 0.0)
            nc.sync.dma_start(ccw_in[:], warm_sb[:])
            nc.gpsimd.collective_compute(
                "AllToAll",
                mybir.AluOpType.bypass,
                replica_groups=rg,
                ins=[ccw_in.opt()],
                outs=[ccw_out.opt()],
            )

            qpair = [
                qkpool.tile([P, S], BF16, tag=f"qpair{m}", name=f"qpair{m}")
                for m in range(2)
            ]
            kdup = qkpool.tile([P, S], BF16, tag="kdup")
            v_aug = qkpool.tile([P, NKB * 65], BF16, tag="v_aug")
            cc_sb = qkpool.tile([P, KT * SROWS], BF16, tag="cc_sb")

            # ========== Phase 1: QKV projection (transposed) + RoPE ==========
            with (
                tc.tile_pool(name="ph1", bufs=1) as ph1,
                tc.tile_pool(name="qkvf", bufs=1) as qkvf,
                tc.tile_pool(name="ropetmp", bufs=1) as ropetmp,
                tc.tile_pool(name="ps_mm", bufs=2, space="PSUM") as ps_mm,
            ):
                # DMA priority: weights + hidden k-tiles first (feed PE),
                # rope/mask constants after.
                wq_sb = ph1.tile([P, KT * 384], BF16, tag="wq_sb")
                nc.sync.dma_start(wq_sb[:], wq_d[:])
                h_tiles = []
                for kt in range(KT):
                    ht = ph1.tile([P, S], BF16, tag=f"h{kt}", name=f"h{kt}")
                    nc.sync.dma_start(ht[:], hT_d[:, kt * S : (kt + 1) * S])
                    h_tiles.append(ht)
                ropec = const.tile([P, S], F32, tag="ropec")
                nc.sync.dma_start(ropec[:], ropec_d[:])
                ropes = const.tile([P, S], F32, tag="ropes")
                nc.sync.dma_start(ropes[:], ropes_d[:])
                ident = const.tile([P, P], F32, tag="ident")
                nc.sync.dma_start(ident[:], ident_d[:])
                masks = const.tile([P, 4 * 2 * NQ], BF16, tag="masks")
                nc.sync.dma_start(masks[:], masks_d[:])

                qkv_sb = qkvf.tile([P, 3 * S], F32, tag="qkv_sb")

                def emit_qkv_group(m, n):
                    ps = ps_mm.tile([P, NQ], F32, tag="mm", name="mmps")
                    for kt in range(KT):
                        nc.tensor.matmul(
                            ps[:],
                            wq_sb[:, kt * 384 + 128 * m : kt * 384 + 128 * (m + 1)],
                            h_tiles[kt][:, NQ * n : NQ * (n + 1)],
                            start=(kt == 0),
                            stop=(kt == KT - 1),
                        )
                    nc.vector.tensor_copy(
                        qkv_sb[:, m * S + NQ * n : m * S + NQ * (n + 1)], ps[:]
                    )

                def emit_rope_q(m):
                    swp = ropetmp.tile([P, S], F32, tag="swp", name="swp")
                    for b in range(2):
                        o = 64 * b
                        nc.vector.tensor_copy(
                            swp[o : o + 32, :],
                            qkv_sb[o + 32 : o + 64, m * S : (m + 1) * S],
                        )
                        nc.vector.tensor_copy(
                            swp[o + 32 : o + 64, :],
                            qkv_sb[o : o + 32, m * S : (m + 1) * S],
                        )
                    prod = ropetmp.tile([P, S], F32, tag="prod", name="prod")
                    nc.vector.tensor_mul(
                        prod[:], qkv_sb[:, m * S : (m + 1) * S], ropec[:]
                    )
                    prod2 = ropetmp.tile([P, S], F32, tag="prod2", name="prod2")
                    nc.vector.tensor_mul(prod2[:], swp[:], ropes[:])
                    nc.vector.tensor_add(qpair[m][:], prod[:], prod2[:])

                # kv block (m=2) first so rope-k / v-transpose / attention can
                # begin while q projections still run.
                for m in (2, 0, 1):
                    for n in range(NJC):
                        emit_qkv_group(m, n)
                    if m in (0, 1):
                        emit_rope_q(m)
                    else:
                        # rope k (partitions 0..63 of kv block)
                        swpk = ropetmp.tile([P, S], F32, tag="swp")
                        nc.vector.tensor_copy(
                            swpk[0:32, :], qkv_sb[32:64, 2 * S : 3 * S]
                        )
                        nc.vector.tensor_copy(
                            swpk[32:64, :], qkv_sb[0:32, 2 * S : 3 * S]
                        )
                        prodk = ropetmp.tile([P, S], F32, tag="prod")
                        nc.vector.tensor_mul(
                            prodk[0:64, :], qkv_sb[0:64, 2 * S : 3 * S], ropec[0:64, :]
                        )
                        prodk2 = ropetmp.tile([P, S], F32, tag="prod2")
                        nc.vector.tensor_mul(
                            prodk2[0:64, :], swpk[0:64, :], ropes[0:64, :]
                        )
                        nc.vector.tensor_add(
                            kdup[0:64, :], prodk[0:64, :], prodk2[0:64, :]
                        )
                        nc.vector.tensor_add(
                            kdup[64:128, :], prodk[0:64, :], prodk2[0:64, :]
                        )
                        # transpose v -> v_aug [keys, 64 | 1]
                        for kb in range(NKB):
                            vps = ps_mm.tile([P, 64], F32, tag="mm")
                            nc.tensor.transpose(
                                vps[:, 0:64],
                                qkv_sb[64:128, 2 * S + P * kb : 2 * S + P * (kb + 1)],
                                ident[64:128, 64:128],
                            )
                            nc.vector.tensor_copy(
                                v_aug[:, kb * 65 : kb * 65 + 64], vps[:, 0:64]
                            )
                            nc.vector.memset(
                                v_aug[:, kb * 65 + 64 : kb * 65 + 65], 1.0
                            )

            # ========== Phase 2: attention per head pair ==========
            cc_insts = []
            last_ccin = [None, None]
            with (
                tc.tile_pool(name="ps_s", bufs=3, space="PSUM") as ps_s,
                tc.tile_pool(name="ps_ctx", bufs=2, space="PSUM") as ps_ctx,
            ):
                for p in range(2):
                    for jc in range(NJC):
                        nkb = 4 * (jc + 1)
                        ctxs = [
                            ps_ctx.tile([P, NQ], F32, tag=f"ctx{hh}", name=f"ctx{hh}")
                            for hh in range(2)
                        ]
                        for kb in range(nkb):
                            d = kb - 4 * jc
                            sps = []
                            for hh in range(2):
                                base = 64 * hh
                                sp = ps_s.tile([P, NQ], F32, tag="sp", name="sp")
                                nc.tensor.matmul(
                                    sp[:],
                                    kdup[base : base + 64, P * kb : P * (kb + 1)],
                                    qpair[p][base : base + 64, NQ * jc : NQ * (jc + 1)],
                                    start=True,
                                    stop=True,
                                )
                                sps.append(sp)
                            for hh in range(2):
                                e = esb.tile([P, NQ], BF16, tag="e", name="e")
                                nc.scalar.activation(
                                    e[:], sps[hh][:],
                                    mybir.ActivationFunctionType.Exp,
                                    scale=0.125,
                                )
                                if d >= 0:
                                    nc.vector.tensor_mul(
                                        e[:], e[:],
                                        masks[:, 2 * NQ * d : 2 * NQ * d + NQ],
                                    )
                                nc.tensor.matmul(
                                    ctxs[hh][0:65, :],
                                    v_aug[:, kb * 65 : kb * 65 + 65],
                                    e[:],
                                    start=(kb == 0),
                                    stop=(kb == nkb - 1),
                                )
                        for hh in range(2):
                            h = 2 * p + hh
                            # reciprocal_approx_fast misreads PSUM at nonzero
                            # base partition -- stage the sum row through SBUF
                            rin = small.tile([1, NQ], F32, tag="rin")
                            nc.vector.tensor_copy(rin[:], ctxs[hh][64:65, :])
                            rec = small.tile([1, NQ], F32, tag="rec")
                            nc.vector.reciprocal_approx_fast(rec[:], rin[:])
                            bcs = small.tile([64, NQ], F32, tag="bcs")
                            nc.gpsimd.partition_broadcast(
                                bcs[:], rec[:], channels=64
                            )
                            ctxn = small.tile([64, NQ], BF16, tag="ctxn")
                            nc.vector.tensor_mul(ctxn[:], ctxs[hh][0:64, :], bcs[:])
                            # scatter: shard j (rows 128j..) holds my pair-p
                            # ctxT rows [64*hh ..] for core j's seq cols
                            for half in range(2):
                                j = 2 * jc + half
                                last_ccin[p] = nc.sync.dma_start(
                                    cc_in[p][
                                        P * j + 64 * hh : P * j + 64 * (hh + 1), :
                                    ],
                                    ctxn[:, SROWS * half : SROWS * (half + 1)],
                                )
                    # A2A for this pair; pair 0's collective overlaps pair 1's
                    # attention.
                    cc_insts.append(
                        nc.gpsimd.collective_compute(
                            "AllToAll",
                            mybir.AluOpType.bypass,
                            replica_groups=rg,
                            ins=[cc_in[p].opt()],
                            outs=[cc_out[p].opt()],
                        )
                    )

            # ========== Phase 3: o_proj on own seq rows ==========
            # qd chunk (2j + p) <- cc_out[p] rows [128j .. 128j+128)
            for j in range(NCORES):
                for p in range(2):
                    kt = 2 * j + p
                    dma = nc.sync.dma_start(
                        cc_sb[:, kt * SROWS : (kt + 1) * SROWS],
                        cc_out[p][j * P : (j + 1) * P, :],
                    )
                    # prevent Sync-queue head-of-line blocking: this DMA waits
                    # on collective completion, so don't let the scheduler
                    # hoist it ahead of pair-1's scatter traffic
                    _br.add_dep_helper(
                        dma.ins, last_ccin[1].ins, sync=True,
                        reason="cc_sb read after pair-1 scatter traffic",
                    )
            with (
                tc.tile_pool(name="wo_stream", bufs=4) as wo_stream,
                tc.tile_pool(name="ps_o", bufs=1, space="PSUM") as ps_o,
            ):
                # 8 PSUM banks, one per (n, m); even-kt chunks only need
                # cc_out[0] so this pass hides under the second AllToAll.
                wo_tiles = []
                for n in range(NJC):
                    wos = wo_stream.tile([P, KT * NQ], BF16, tag="wos", name=f"wos{n}")
                    nc.sync.dma_start(
                        wos[:].rearrange("p (kt c) -> p kt c", kt=KT),
                        wo_d[:].rearrange("p (kt c) -> p kt c", kt=KT)[
                            :, :, NQ * n : NQ * (n + 1)
                        ],
                    )
                    wo_tiles.append(wos)
                o_ps = {}
                for n in range(NJC):
                    for m in range(2):
                        o_ps[(n, m)] = ps_o.tile(
                            [P, NQ], F32, tag=f"o{n}{m}", name=f"o{n}{m}"
                        )
                for parity in range(2):
                    for n in range(NJC):
                        for m in range(2):
                            for kk in range(KT // 2):
                                kt = 2 * kk + parity
                                nc.tensor.matmul(
                                    o_ps[(n, m)][:],
                                    cc_sb[
                                        :, kt * SROWS + P * m : kt * SROWS + P * (m + 1)
                                    ],
                                    wo_tiles[n][:, kt * NQ : (kt + 1) * NQ],
                                    start=(parity == 0 and kk == 0),
                                    stop=(parity == 1 and kk == KT // 2 - 1),
                                )
                for n in range(NJC):
                    for m in range(2):
                        ot = outp.tile([P, NQ], F32, tag="ot")
                        nc.vector.tensor_copy(ot[:], o_ps[(n, m)][:])
                        nc.sync.dma_start(
                            out_d[P * m : P * (m + 1), NQ * n : NQ * (n + 1)], ot[:]
                        )

    nc.compile()
    return nc


def _get_nc():
    global _NC_CACHE
    if _NC_CACHE is None:
        _NC_CACHE = _build_nc()
    return _NC_CACHE


def _stage_inputs(position_ids, hidden_states, w_qkv, w_o):
    """Host-side sharding / layout staging. Returns in_maps for 8 cores."""
    pos = np.asarray(position_ids)[0].astype(np.float32)            # [S]
    hidden = np.asarray(hidden_states, dtype=np.float32)[0]         # [S, H]
    w_qkv = np.asarray(w_qkv, dtype=np.float32)                     # [3072, H]
    w_o = np.asarray(w_o, dtype=np.float32)                         # [H, Q_SIZE]

    # hT tiles: [H, S] -> [128, KT*S] (k-tile kt at cols kt*S..)
    hT = np.ascontiguousarray(hidden.T)
    hT_r = np.ascontiguousarray(
        hT.reshape(KT, P, S).transpose(1, 0, 2).reshape(P, KT * S)
    ).astype(BF16_NP)

    # w_o.T tiles: [Q_SIZE, H] -> [128, KT*H]
    woT = np.ascontiguousarray(w_o.T)
    woT_r = np.ascontiguousarray(
        woT.reshape(KT, P, H).transpose(1, 0, 2).reshape(P, KT * H)
    ).astype(BF16_NP)

    # rope tables in [d, s] layout for a [128 = 2 heads x 64] tile
    inv_freq = (1.0 / (ROPE_THETA ** (np.arange(0, HD, 2, dtype=np.float32) / HD)))
    ang = pos[:, None] * inv_freq[None, :]                          # [S, 32]
    cosT = np.cos(ang).T.astype(np.float32)                         # [32, S]
    sinT = np.sin(ang).T.astype(np.float32)
    ropec = np.concatenate([cosT, cosT, cosT, cosT], axis=0)        # [128, S]
    ropes = np.concatenate([-sinT, sinT, -sinT, sinT], axis=0)      # [128, S]

    # causal masks for the 4 diagonal block offsets, stored duplicated
    # (legacy layout: [128, 8*NQ], block d at cols 2*NQ*d, first NQ used)
    f = np.arange(NQ)
    mask_list = []
    for d in range(4):
        pp = np.arange(P)[:, None] + 128 * d
        mk = (pp <= f[None, :]).astype(BF16_NP)
        mask_list.append(mk)
        mask_list.append(mk)
    masks = np.concatenate(mask_list, axis=1)                       # [128, 8*NQ]

    ident = np.eye(P, dtype=np.float32)

    in_maps = []
    for i in range(NCORES):
        rows_q = w_qkv[QH * HD * i : QH * HD * (i + 1)]             # [256, H]
        row_k = w_qkv[Q_SIZE + HD * i : Q_SIZE + HD * (i + 1)]      # [64, H]
        row_v = w_qkv[Q_SIZE + KV_SIZE + HD * i : Q_SIZE + KV_SIZE + HD * (i + 1)]
        wshard = np.concatenate([rows_q, row_k, row_v], axis=0)     # [384, H]
        wqkvT = np.ascontiguousarray(wshard.T)                      # [H, 384]
        wqkvT_r = np.ascontiguousarray(
            wqkvT.reshape(KT, P, 384).transpose(1, 0, 2).reshape(P, KT * 384)
        ).astype(BF16_NP)
        in_maps.append(
            {
                "hT": hT_r,
                "wqkvT": wqkvT_r,
                "woT": woT_r,
                "ropec": ropec,
                "ropes": ropes,
                "masks": masks,
                "ident": ident,
            }
        )
    return in_maps


def _ensure_ntff_hook():
    """The container's antenv stub lacks axon_hooks, so trn_boot silently
    skipped NTFF hook registration. Recreate the module and register the
    ctypes-based hook so run_bass_kernel_spmd(trace=True) can profile."""
    import sys
    import types

    if "antenv.axon_hooks" in sys.modules:
        return
    try:
        import antenv
        from trn_agent_boot.trn_boot import _ntff_profile_via_ctypes

        hooks = types.ModuleType("antenv.axon_hooks")
        _state = {}

        def set_axon_ntff_profile_hook(h):
            _state["h"] = h

        def get_axon_ntff_profile_hook():
            return _state.get("h")

        hooks.set_axon_ntff_profile_hook = set_axon_ntff_profile_hook
        hooks.get_axon_ntff_profile_hook = get_axon_ntff_profile_hook
        sys.modules["antenv.axon_hooks"] = hooks
        antenv.axon_hooks = hooks
        hook = _ntff_profile_via_ctypes("/opt/axon/libaxon_pjrt.so")
        if hook is not None:
            set_axon_ntff_profile_hook(hook)
    except Exception:
        pass


def kernel(**inputs):
    global LAST_RESULTS
    from concourse.bass_utils import run_bass_kernel_spmd

    nc = _get_nc()
    in_maps = _stage_inputs(
        inputs["position_ids"], inputs["hidden_states"], inputs["w_qkv"], inputs["w_o"]
    )
    trace = os.environ.get("KERNEL_TRACE", "0") == "1"
    if trace:
        _ensure_ntff_hook()
    res = run_bass_kernel_spmd(
        nc, in_maps, core_ids=list(range(NCORES)), trace=trace
    )
    LAST_RESULTS = res
    outs = [np.asarray(res.results[i]["out"], dtype=np.float32) for i in range(NCORES)]
    full = np.concatenate(outs, axis=0)                             # [S, H]
    return full.reshape(1, S, H)


# revision 15
# speedup vs baseline: 1.5825x; 1.2546x over previous
"""Distributed GQA attention kernel for 8 TRN2 NeuronCores.

Strategy (tensor-parallel over heads, A2A re-shard before o_proj):
  - Core i owns q heads 4i..4i+3 and kv head i (GQA group) -> attention is
    fully local per core.
  - QKV projection computed TRANSPOSED (features on partitions):
      qkvT[f, s] = w_qkv_shard @ hidden.T   (lhsT = w_qkv_shard.T, rhs = hidden.T)
  - RoPE (neox) applied in [d, s] layout via elementwise DVE ops.
  - Attention in "scoresT" layout: scoresT[k, q] = kT.T @ qT (K=d=64 contraction,
    row-packed head pairs at base partitions 0/64 run concurrently on the PE).
    Softmax without max-subtraction (scores are O(1)); exp on ACT; causal
    masking via 0/1 mask multiply after exp on diagonal blocks; PV matmul with
    ones-augmented v gives both ctxT and the softmax denominators in one
    accumulation; denominators broadcast via gpsimd.partition_broadcast.
  - Two AllToAlls (one per head pair) re-shard ctxT from head-sharded to
    sequence-sharded; the first overlaps the second pair's attention; a tiny
    warm-up collective at kernel start absorbs the ncfw cold-start delay.
  - o_proj locally on own 256 seq rows with full w_o.T, split into an even-kt
    pass (only needs the first A2A) that hides under the second A2A, then an
    odd-kt pass. Output row-sharded, host concatenates.
"""

import os
import numpy as np
import ml_dtypes

import concourse.bass as bass
import concourse.mybir as mybir
from concourse import bacc, tile
import bass_rust as _br

F32 = mybir.dt.float32
BF16 = mybir.dt.bfloat16
BF16_NP = ml_dtypes.bfloat16

# Problem constants (hardcoded per harness contract)
H = 2048
S = 2048
NH = 32
NKV = 8
HD = 64
Q_SIZE = NH * HD      # 2048
KV_SIZE = NKV * HD    # 512
NCORES = 8
QH = NH // NCORES     # 4 q heads per core
ROPE_THETA = 10000.0

P = 128
KT = H // P           # 16 contraction tiles over H
NQ = 512              # query chunk (matmul moving free dim)
NJC = S // NQ         # 4 query chunks
NKB = S // P          # 16 key tiles
SROWS = S // NCORES   # 256 seq rows per core after A2A

_NC_CACHE = None
LAST_RESULTS = None


def _build_nc():
    nc = bacc.Bacc(
        "TRN2",
        target_bir_lowering=False,
        debug=False,
        num_devices=NCORES,
    )

    # ---- I/O ----
    hT_d = nc.dram_tensor("hT", [P, KT * S], BF16, kind="ExternalInput")
    wq_d = nc.dram_tensor("wqkvT", [P, KT * 384], BF16, kind="ExternalInput")
    wo_d = nc.dram_tensor("woT", [P, KT * H], BF16, kind="ExternalInput")
    ropec_d = nc.dram_tensor("ropec", [P, S], F32, kind="ExternalInput")
    ropes_d = nc.dram_tensor("ropes", [P, S], F32, kind="ExternalInput")
    masks_d = nc.dram_tensor("masks", [P, 4 * 2 * NQ], BF16, kind="ExternalInput")
    ident_d = nc.dram_tensor("ident", [P, P], F32, kind="ExternalInput")
    out_d = nc.dram_tensor("out", [SROWS, H], F32, kind="ExternalOutput")

    rg = [list(range(NCORES))]

    with tile.TileContext(nc) as tc:
        with (
            tc.tile_pool(name="dram", bufs=1, space="DRAM") as dram,
            tc.tile_pool(name="const", bufs=1) as const,
            tc.tile_pool(name="qk", bufs=1) as qkpool,
            tc.tile_pool(name="esb", bufs=4) as esb,
            tc.tile_pool(name="small", bufs=2) as small,
            tc.tile_pool(name="outp", bufs=2) as outp,
        ):
            # A2A buffers, one per head pair (shard j rows = my pair ctxT for
            # seq cols of core j)
            cc_in = [
                dram.tile([NCORES * P, SROWS], BF16, tag=f"cc_in{p}", name=f"cc_in{p}")
                for p in range(2)
            ]
            cc_out = [
                dram.tile([NCORES * P, SROWS], BF16, tag=f"cc_out{p}", name=f"cc_out{p}")
                for p in range(2)
            ]

            # tiny warm-up collective absorbs the ~11.5us ncfw first-collective
            # start delay so the real A2As get the warm path
            ccw_in = dram.tile([NCORES, 16], BF16, tag="ccw_in")
            ccw_out = dram.tile([NCORES, 16], BF16, tag="ccw_out")
            warm_sb = const.tile([NCORES, 16], BF16, tag="warm_sb")
            nc.vector.memset(warm_sb[:], 0.0)
            nc.sync.dma_start(ccw_in[:], warm_sb[:])
            nc.gpsimd.collective_compute(
                "AllToAll",
                mybir.AluOpType.bypass,
                replica_groups=rg,
                ins=[ccw_in.opt()],
                outs=[ccw_out.opt()],
            )

            qpair = [
                qkpool.tile([P, S], BF16, tag=f"qpair{m}", name=f"qpair{m}")
                for m in range(2)
            ]
            kdup = qkpool.tile([P, S], BF16, tag="kdup")
            v_aug = qkpool.tile([P, NKB * 65], BF16, tag="v_aug")
            cc_sb = qkpool.tile([P, KT * SROWS], BF16, tag="cc_sb")

            # ========== Phase 1: QKV projection (transposed) + RoPE ==========
            with (
                tc.tile_pool(name="ph1", bufs=1) as ph1,
                tc.tile_pool(name="qkvf", bufs=1) as qkvf,
                tc.tile_pool(name="ropetmp", bufs=1) as ropetmp,
                tc.tile_pool(name="ps_mm", bufs=2, space="PSUM") as ps_mm,
            ):
                # DMA priority: weights + hidden k-tiles first (feed PE),
                # rope/mask constants after.
                wq_sb = ph1.tile([P, KT * 384], BF16, tag="wq_sb")
                nc.sync.dma_start(wq_sb[:], wq_d[:])
                h_tiles = []
                for kt in range(KT):
                    ht = ph1.tile([P, S], BF16, tag=f"h{kt}", name=f"h{kt}")
                    nc.sync.dma_start(ht[:], hT_d[:, kt * S : (kt + 1) * S])
                    h_tiles.append(ht)
                ropec = const.tile([P, S], F32, tag="ropec")
                nc.sync.dma_start(ropec[:], ropec_d[:])
                ropes = const.tile([P, S], F32, tag="ropes")
                nc.sync.dma_start(ropes[:], ropes_d[:])
                ident = const.tile([P, P], F32, tag="ident")
                nc.sync.dma_start(ident[:], ident_d[:])
                masks = const.tile([P, 4 * 2 * NQ], BF16, tag="masks")
                nc.sync.dma_start(masks[:], masks_d[:])

                qkv_sb = qkvf.tile([P, 3 * S], F32, tag="qkv_sb")

                def emit_qkv_group(m, n):
                    ps = ps_mm.tile([P, NQ], F32, tag="mm", name="mmps")
                    for kt in range(KT):
                        nc.tensor.matmul(
                            ps[:],
                            wq_sb[:, kt * 384 + 128 * m : kt * 384 + 128 * (m + 1)],
                            h_tiles[kt][:, NQ * n : NQ * (n + 1)],
                            start=(kt == 0),
                            stop=(kt == KT - 1),
                        )
                    nc.vector.tensor_copy(
                        qkv_sb[:, m * S + NQ * n : m * S + NQ * (n + 1)], ps[:]
                    )

                def emit_rope_q(m):
                    swp = ropetmp.tile([P, S], F32, tag="swp", name="swp")
                    for b in range(2):
                        o = 64 * b
                        nc.vector.tensor_copy(
                            swp[o : o + 32, :],
                            qkv_sb[o + 32 : o + 64, m * S : (m + 1) * S],
                        )
                        nc.vector.tensor_copy(
                            swp[o + 32 : o + 64, :],
                            qkv_sb[o : o + 32, m * S : (m + 1) * S],
                        )
                    prod = ropetmp.tile([P, S], F32, tag="prod", name="prod")
                    nc.vector.tensor_mul(
                        prod[:], qkv_sb[:, m * S : (m + 1) * S], ropec[:]
                    )
                    prod2 = ropetmp.tile([P, S], F32, tag="prod2", name="prod2")
                    nc.vector.tensor_mul(prod2[:], swp[:], ropes[:])
                    nc.vector.tensor_add(qpair[m][:], prod[:], prod2[:])

                # kv block (m=2) first so rope-k / v-transpose / attention can
                # begin while q projections still run.
                for m in (2, 0, 1):
                    for n in range(NJC):
                        emit_qkv_group(m, n)
                    if m in (0, 1):
                        emit_rope_q(m)
                    else:
                        # rope k (partitions 0..63 of kv block)
                        swpk = ropetmp.tile([P, S], F32, tag="swp")
                        nc.vector.tensor_copy(
                            swpk[0:32, :], qkv_sb[32:64, 2 * S : 3 * S]
                        )
                        nc.vector.tensor_copy(
                            swpk[32:64, :], qkv_sb[0:32, 2 * S : 3 * S]
                        )
                        prodk = ropetmp.tile([P, S], F32, tag="prod")
                        nc.vector.tensor_mul(
                            prodk[0:64, :], qkv_sb[0:64, 2 * S : 3 * S], ropec[0:64, :]
                        )
                        prodk2 = ropetmp.tile([P, S], F32, tag="prod2")
                        nc.vector.tensor_mul(
                            prodk2[0:64, :], swpk[0:64, :], ropes[0:64, :]
                        )
                        nc.vector.tensor_add(
                            kdup[0:64, :], prodk[0:64, :], prodk2[0:64, :]
                        )
                        nc.vector.tensor_add(
                            kdup[64:128, :], prodk[0:64, :], prodk2[0:64, :]
                        )
                        # transpose v -> v_aug [keys, 64 | 1]
                        for kb in range(NKB):
                            vps = ps_mm.tile([P, 64], F32, tag="mm")
                            nc.tensor.transpose(
                                vps[:, 0:64],
                                qkv_sb[64:128, 2 * S + P * kb : 2 * S + P * (kb + 1)],
                                ident[64:128, 64:128],
                            )
                            nc.vector.tensor_copy(
                                v_aug[:, kb * 65 : kb * 65 + 64], vps[:, 0:64]
                            )
                            nc.vector.memset(
                                v_aug[:, kb * 65 + 64 : kb * 65 + 65], 1.0
                            )

            # ========== Phase 2: attention per head pair ==========
            cc_insts = []
            last_ccin = [None, None]
            with (
                tc.tile_pool(name="ps_s", bufs=2, space="PSUM") as ps_s,
                tc.tile_pool(name="ps_ctx", bufs=2, space="PSUM") as ps_ctx,
            ):
                for p in range(2):
                    for jc in range(NJC):
                        nkb = 4 * (jc + 1)
                        ctxs = [
                            ps_ctx.tile([P, NQ], F32, tag=f"ctx{hh}", name=f"ctx{hh}")
                            for hh in range(2)
                        ]
                        for kb in range(nkb):
                            d = kb - 4 * jc
                            sp = ps_s.tile([P, 2 * NQ], F32, tag="sp", name="sp")
                            for hh in range(2):
                                base = 64 * hh
                                nc.tensor.matmul(
                                    sp[:, NQ * hh : NQ * (hh + 1)],
                                    kdup[base : base + 64, P * kb : P * (kb + 1)],
                                    qpair[p][base : base + 64, NQ * jc : NQ * (jc + 1)],
                                    start=True,
                                    stop=True,
                                )
                            e = esb.tile([P, 2 * NQ], BF16, tag="e", name="e")
                            nc.scalar.activation(
                                e[:], sp[:], mybir.ActivationFunctionType.Exp,
                                scale=0.125,
                            )
                            if d >= 0:
                                nc.vector.tensor_mul(
                                    e[:], e[:],
                                    masks[:, 2 * NQ * d : 2 * NQ * (d + 1)],
                                )
                            for hh in range(2):
                                nc.tensor.matmul(
                                    ctxs[hh][0:65, :],
                                    v_aug[:, kb * 65 : kb * 65 + 65],
                                    e[:, NQ * hh : NQ * (hh + 1)],
                                    start=(kb == 0),
                                    stop=(kb == nkb - 1),
                                )
                        for hh in range(2):
                            h = 2 * p + hh
                            # reciprocal_approx_fast misreads PSUM at nonzero
                            # base partition -- stage the sum row through SBUF
                            rin = small.tile([1, NQ], F32, tag="rin")
                            nc.vector.tensor_copy(rin[:], ctxs[hh][64:65, :])
                            rec = small.tile([1, NQ], F32, tag="rec")
                            nc.vector.reciprocal_approx_fast(rec[:], rin[:])
                            bcs = small.tile([64, NQ], F32, tag="bcs")
                            nc.gpsimd.partition_broadcast(
                                bcs[:], rec[:], channels=64
                            )
                            ctxn = small.tile([64, NQ], BF16, tag="ctxn")
                            nc.vector.tensor_mul(ctxn[:], ctxs[hh][0:64, :], bcs[:])
                            # scatter: shard j (rows 128j..) holds my pair-p
                            # ctxT rows [64*hh ..] for core j's seq cols
                            for half in range(2):
                                j = 2 * jc + half
                                last_ccin[p] = nc.sync.dma_start(
                                    cc_in[p][
                                        P * j + 64 * hh : P * j + 64 * (hh + 1), :
                                    ],
                                    ctxn[:, SROWS * half : SROWS * (half + 1)],
                                )
                    # A2A for this pair; pair 0's collective overlaps pair 1's
                    # attention.
                    cc_insts.append(
                        nc.gpsimd.collective_compute(
                            "AllToAll",
                            mybir.AluOpType.bypass,
                            replica_groups=rg,
                            ins=[cc_in[p].opt()],
                            outs=[cc_out[p].opt()],
                        )
                    )

            # ========== Phase 3: o_proj on own seq rows ==========
            # qd chunk (2j + p) <- cc_out[p] rows [128j .. 128j+128)
            for j in range(NCORES):
                for p in range(2):
                    kt = 2 * j + p
                    dma = nc.sync.dma_start(
                        cc_sb[:, kt * SROWS : (kt + 1) * SROWS],
                        cc_out[p][j * P : (j + 1) * P, :],
                    )
                    # prevent Sync-queue head-of-line blocking: this DMA waits
                    # on collective completion, so don't let the scheduler
                    # hoist it ahead of pair-1's scatter traffic
                    _br.add_dep_helper(
                        dma.ins, last_ccin[1].ins, sync=True,
                        reason="cc_sb read after pair-1 scatter traffic",
                    )
            with (
                tc.tile_pool(name="wo_stream", bufs=4) as wo_stream,
                tc.tile_pool(name="ps_o", bufs=1, space="PSUM") as ps_o,
            ):
                # 8 PSUM banks, one per (n, m); even-kt chunks only need
                # cc_out[0] so this pass hides under the second AllToAll.
                wo_tiles = []
                for n in range(NJC):
                    wos = wo_stream.tile([P, KT * NQ], BF16, tag="wos", name=f"wos{n}")
                    nc.sync.dma_start(
                        wos[:].rearrange("p (kt c) -> p kt c", kt=KT),
                        wo_d[:].rearrange("p (kt c) -> p kt c", kt=KT)[
                            :, :, NQ * n : NQ * (n + 1)
                        ],
                    )
                    wo_tiles.append(wos)
                o_ps = {}
                for n in range(NJC):
                    for m in range(2):
                        o_ps[(n, m)] = ps_o.tile(
                            [P, NQ], F32, tag=f"o{n}{m}", name=f"o{n}{m}"
                        )
                for parity in range(2):
                    for n in range(NJC):
                        for m in range(2):
                            for kk in range(KT // 2):
                                kt = 2 * kk + parity
                                nc.tensor.matmul(
                                    o_ps[(n, m)][:],
                                    cc_sb[
                                        :, kt * SROWS + P * m : kt * SROWS + P * (m + 1)
                                    ],
                                    wo_tiles[n][:, kt * NQ : (kt + 1) * NQ],
                                    start=(parity == 0 and kk == 0),
                                    stop=(parity == 1 and kk == KT // 2 - 1),
                                )
                for n in range(NJC):
                    for m in range(2):
                        ot = outp.tile([P, NQ], F32, tag="ot")
                        nc.vector.tensor_copy(ot[:], o_ps[(n, m)][:])
                        nc.sync.dma_start(
                            out_d[P * m : P * (m + 1), NQ * n : NQ * (n + 1)], ot[:]
                        )

    nc.compile()
    return nc


def _get_nc():
    global _NC_CACHE
    if _NC_CACHE is None:
        _NC_CACHE = _build_nc()
    return _NC_CACHE


def _stage_inputs(position_ids, hidden_states, w_qkv, w_o):
    """Host-side sharding / layout staging. Returns in_maps for 8 cores."""
    pos = np.asarray(position_ids)[0].astype(np.float32)            # [S]
    hidden = np.asarray(hidden_states, dtype=np.float32)[0]         # [S, H]
    w_qkv = np.asarray(w_qkv, dtype=np.float32)                     # [3072, H]
    w_o = np.asarray(w_o, dtype=np.float32)                         # [H, Q_SIZE]

    # hT tiles: [H, S] -> [128, KT*S] (k-tile kt at cols kt*S..)
    hT = np.ascontiguousarray(hidden.T)
    hT_r = np.ascontiguousarray(
        hT.reshape(KT, P, S).transpose(1, 0, 2).reshape(P, KT * S)
    ).astype(BF16_NP)

    # w_o.T tiles: [Q_SIZE, H] -> [128, KT*H]
    woT = np.ascontiguousarray(w_o.T)
    woT_r = np.ascontiguousarray(
        woT.reshape(KT, P, H).transpose(1, 0, 2).reshape(P, KT * H)
    ).astype(BF16_NP)

    # rope tables in [d, s] layout for a [128 = 2 heads x 64] tile
    inv_freq = (1.0 / (ROPE_THETA ** (np.arange(0, HD, 2, dtype=np.float32) / HD)))
    ang = pos[:, None] * inv_freq[None, :]                          # [S, 32]
    cosT = np.cos(ang).T.astype(np.float32)                         # [32, S]
    sinT = np.sin(ang).T.astype(np.float32)
    ropec = np.concatenate([cosT, cosT, cosT, cosT], axis=0)        # [128, S]
    ropes = np.concatenate([-sinT, sinT, -sinT, sinT], axis=0)      # [128, S]

    # causal masks for the 4 diagonal block offsets, stored duplicated
    # (legacy layout: [128, 8*NQ], block d at cols 2*NQ*d, first NQ used)
    f = np.arange(NQ)
    mask_list = []
    for d in range(4):
        pp = np.arange(P)[:, None] + 128 * d
        mk = (pp <= f[None, :]).astype(BF16_NP)
        mask_list.append(mk)
        mask_list.append(mk)
    masks = np.concatenate(mask_list, axis=1)                       # [128, 8*NQ]

    ident = np.eye(P, dtype=np.float32)

    in_maps = []
    for i in range(NCORES):
        rows_q = w_qkv[QH * HD * i : QH * HD * (i + 1)]             # [256, H]
        row_k = w_qkv[Q_SIZE + HD * i : Q_SIZE + HD * (i + 1)]      # [64, H]
        row_v = w_qkv[Q_SIZE + KV_SIZE + HD * i : Q_SIZE + KV_SIZE + HD * (i + 1)]
        wshard = np.concatenate([rows_q, row_k, row_v], axis=0)     # [384, H]
        wqkvT = np.ascontiguousarray(wshard.T)                      # [H, 384]
        wqkvT_r = np.ascontiguousarray(
            wqkvT.reshape(KT, P, 384).transpose(1, 0, 2).reshape(P, KT * 384)
        ).astype(BF16_NP)
        in_maps.append(
            {
                "hT": hT_r,
                "wqkvT": wqkvT_r,
                "woT": woT_r,
                "ropec": ropec,
                "ropes": ropes,
                "masks": masks,
                "ident": ident,
            }
        )
    return in_maps


def _ensure_ntff_hook():
    """The container's antenv stub lacks axon_hooks, so trn_boot silently
    skipped NTFF hook registration. Recreate the module and register the
    ctypes-based hook so run_bass_kernel_spmd(trace=True) can profile."""
    import sys
    import types

    if "antenv.axon_hooks" in sys.modules:
        return
    try:
        import antenv
        from trn_agent_boot.trn_boot import _ntff_profile_via_ctypes

        hooks = types.ModuleType("antenv.axon_hooks")
        _state = {}

        def set_axon_ntff_profile_hook(h):
            _state["h"] = h

        def get_axon_ntff_profile_hook():
            return _state.get("h")

        hooks.set_axon_ntff_profile_hook = set_axon_ntff_profile_hook
        hooks.get_axon_ntff_profile_hook = get_axon_ntff_profile_hook
        sys.modules["antenv.axon_hooks"] = hooks
        antenv.axon_hooks = hooks
        hook = _ntff_profile_via_ctypes("/opt/axon/libaxon_pjrt.so")
        if hook is not None:
            set_axon_ntff_profile_hook(hook)
    except Exception:
        pass


def kernel(**inputs):
    global LAST_RESULTS
    from concourse.bass_utils import run_bass_kernel_spmd

    nc = _get_nc()
    in_maps = _stage_inputs(
        inputs["position_ids"], inputs["hidden_states"], inputs["w_qkv"], inputs["w_o"]
    )
    trace = os.environ.get("KERNEL_TRACE", "0") == "1"
    if trace:
        _ensure_ntff_hook()
    res = run_bass_kernel_spmd(
        nc, in_maps, core_ids=list(range(NCORES)), trace=trace
    )
    LAST_RESULTS = res
    outs = [np.asarray(res.results[i]["out"], dtype=np.float32) for i in range(NCORES)]
    full = np.concatenate(outs, axis=0)                             # [S, H]
    return full.reshape(1, S, H)


# revision 18
# speedup vs baseline: 1.5955x; 1.0082x over previous
"""Distributed GQA attention kernel for 8 TRN2 NeuronCores.

Strategy (tensor-parallel over heads, A2A re-shard before o_proj):
  - Core i owns q heads 4i..4i+3 and kv head i (GQA group) -> attention is
    fully local per core.
  - QKV projection computed TRANSPOSED (features on partitions):
      qkvT[f, s] = w_qkv_shard @ hidden.T   (lhsT = w_qkv_shard.T, rhs = hidden.T)
  - RoPE (neox) applied in [d, s] layout via elementwise DVE ops.
  - Attention in "scoresT" layout: scoresT[k, q] = kT.T @ qT (K=d=64 contraction,
    row-packed head pairs at base partitions 0/64 run concurrently on the PE).
    Softmax without max-subtraction (scores are O(1)); exp on ACT; causal
    masking via 0/1 mask multiply after exp on diagonal blocks; PV matmul with
    ones-augmented v gives both ctxT and the softmax denominators in one
    accumulation; denominators broadcast via gpsimd.partition_broadcast.
  - Two AllToAlls (one per head pair) re-shard ctxT from head-sharded to
    sequence-sharded; the first overlaps the second pair's attention; a tiny
    warm-up collective at kernel start absorbs the ncfw cold-start delay.
  - o_proj locally on own 256 seq rows with full w_o.T, split into an even-kt
    pass (only needs the first A2A) that hides under the second A2A, then an
    odd-kt pass. Output row-sharded, host concatenates.
"""

import os
import numpy as np
import ml_dtypes

import concourse.bass as bass
import concourse.mybir as mybir
from concourse import bacc, tile
import bass_rust as _br

F32 = mybir.dt.float32
BF16 = mybir.dt.bfloat16
BF16_NP = ml_dtypes.bfloat16

# Problem constants (hardcoded per harness contract)
H = 2048
S = 2048
NH = 32
NKV = 8
HD = 64
Q_SIZE = NH * HD      # 2048
KV_SIZE = NKV * HD    # 512
NCORES = 8
QH = NH // NCORES     # 4 q heads per core
ROPE_THETA = 10000.0

P = 128
KT = H // P           # 16 contraction tiles over H
NQ = 512              # query chunk (matmul moving free dim)
NJC = S // NQ         # 4 query chunks
NKB = S // P          # 16 key tiles
SROWS = S // NCORES   # 256 seq rows per core after A2A

_NC_CACHE = None
LAST_RESULTS = None


def _build_nc():
    nc = bacc.Bacc(
        "TRN2",
        target_bir_lowering=False,
        debug=False,
        num_devices=NCORES,
    )

    # ---- I/O ----
    hT_d = nc.dram_tensor("hT", [P, KT * S], BF16, kind="ExternalInput")
    wq_d = nc.dram_tensor("wqkvT", [P, KT * 384], BF16, kind="ExternalInput")
    wo_d = nc.dram_tensor("woT", [P, KT * H], BF16, kind="ExternalInput")
    ropec_d = nc.dram_tensor("ropec", [P, S], F32, kind="ExternalInput")
    ropes_d = nc.dram_tensor("ropes", [P, S], F32, kind="ExternalInput")
    masks_d = nc.dram_tensor("masks", [P, 4 * 2 * NQ], BF16, kind="ExternalInput")
    ident_d = nc.dram_tensor("ident", [P, P], F32, kind="ExternalInput")
    out_d = nc.dram_tensor("out", [SROWS, H], F32, kind="ExternalOutput")

    rg = [list(range(NCORES))]

    with tile.TileContext(nc) as tc:
        with (
            tc.tile_pool(name="dram", bufs=1, space="DRAM") as dram,
            tc.tile_pool(name="const", bufs=1) as const,
            tc.tile_pool(name="qk", bufs=1) as qkpool,
            tc.tile_pool(name="esb", bufs=4) as esb,
            tc.tile_pool(name="small", bufs=2) as small,
            tc.tile_pool(name="outp", bufs=2) as outp,
        ):
            # A2A buffers, one per head pair (shard j rows = my pair ctxT for
            # seq cols of core j)
            cc_in = [
                dram.tile([NCORES * P, SROWS], BF16, tag=f"cc_in{p}", name=f"cc_in{p}")
                for p in range(2)
            ]
            cc_out = [
                dram.tile([NCORES * P, SROWS], BF16, tag=f"cc_out{p}", name=f"cc_out{p}")
                for p in range(2)
            ]

            # tiny warm-up collective absorbs the ~11.5us ncfw first-collective
            # start delay so the real A2As get the warm path
            ccw_in = dram.tile([NCORES, 16], BF16, tag="ccw_in")
            ccw_out = dram.tile([NCORES, 16], BF16, tag="ccw_out")
            warm_sb = const.tile([NCORES, 16], BF16, tag="warm_sb")
            nc.vector.memset(warm_sb[:], 0.0)
            nc.sync.dma_start(ccw_in[:], warm_sb[:])
            nc.gpsimd.collective_compute(
                "AllToAll",
                mybir.AluOpType.bypass,
                replica_groups=rg,
                ins=[ccw_in.opt()],
                outs=[ccw_out.opt()],
            )

            qpair = [
                qkpool.tile([P, S], BF16, tag=f"qpair{m}", name=f"qpair{m}")
                for m in range(2)
            ]
            kdup = qkpool.tile([P, S], BF16, tag="kdup")
            v_aug = qkpool.tile([P, NKB * 65], BF16, tag="v_aug")
            cc_sb = qkpool.tile([P, KT * SROWS], BF16, tag="cc_sb")

            # ========== Phase 1: QKV projection (transposed) + RoPE ==========
            with (
                tc.tile_pool(name="ph1", bufs=1) as ph1,
                tc.tile_pool(name="qkvf", bufs=1) as qkvf,
                tc.tile_pool(name="ropetmp", bufs=1) as ropetmp,
                tc.tile_pool(name="ps_mm", bufs=2, space="PSUM") as ps_mm,
            ):
                # DMA priority: weights + hidden k-tiles first (feed PE),
                # rope/mask constants after.
                wq_sb = ph1.tile([P, KT * 384], BF16, tag="wq_sb")
                h_tiles = []
                for kt in range(KT):
                    nc.sync.dma_start(
                        wq_sb[:, kt * 384 : (kt + 1) * 384],
                        wq_d[:, kt * 384 : (kt + 1) * 384],
                    )
                    ht = ph1.tile([P, S], BF16, tag=f"h{kt}", name=f"h{kt}")
                    nc.sync.dma_start(ht[:], hT_d[:, kt * S : (kt + 1) * S])
                    h_tiles.append(ht)
                ropec = const.tile([P, S], F32, tag="ropec")
                nc.sync.dma_start(ropec[:], ropec_d[:])
                ropes = const.tile([P, S], F32, tag="ropes")
                nc.sync.dma_start(ropes[:], ropes_d[:])
                ident = const.tile([P, P], F32, tag="ident")
                nc.sync.dma_start(ident[:], ident_d[:])
                masks = const.tile([P, 4 * 2 * NQ], BF16, tag="masks")
                nc.sync.dma_start(masks[:], masks_d[:])

                qkv_sb = qkvf.tile([P, 3 * S], F32, tag="qkv_sb")

                def emit_qkv_group(m, n):
                    ps = ps_mm.tile([P, NQ], F32, tag="mm", name="mmps")
                    for kt in range(KT):
                        nc.tensor.matmul(
                            ps[:],
                            wq_sb[:, kt * 384 + 128 * m : kt * 384 + 128 * (m + 1)],
                            h_tiles[kt][:, NQ * n : NQ * (n + 1)],
                            start=(kt == 0),
                            stop=(kt == KT - 1),
                        )
                    nc.vector.tensor_copy(
                        qkv_sb[:, m * S + NQ * n : m * S + NQ * (n + 1)], ps[:]
                    )

                def emit_rope_q(m, n):
                    # one NQ-wide chunk so attention jc=n can start as soon as
                    # its q columns are roped
                    c0, c1 = m * S + NQ * n, m * S + NQ * (n + 1)
                    swp = ropetmp.tile([P, NQ], F32, tag="swp", name="swp", bufs=2)
                    for b in range(2):
                        o = 64 * b
                        nc.vector.tensor_copy(
                            swp[o : o + 32, :], qkv_sb[o + 32 : o + 64, c0:c1]
                        )
                        nc.vector.tensor_copy(
                            swp[o + 32 : o + 64, :], qkv_sb[o : o + 32, c0:c1]
                        )
                    prod = ropetmp.tile([P, NQ], F32, tag="prod", name="prod", bufs=2)
                    nc.vector.tensor_mul(
                        prod[:], qkv_sb[:, c0:c1], ropec[:, NQ * n : NQ * (n + 1)]
                    )
                    prod2 = ropetmp.tile(
                        [P, NQ], F32, tag="prod2", name="prod2", bufs=2
                    )
                    nc.vector.tensor_mul(
                        prod2[:], swp[:], ropes[:, NQ * n : NQ * (n + 1)]
                    )
                    nc.vector.tensor_add(
                        qpair[m][:, NQ * n : NQ * (n + 1)], prod[:], prod2[:]
                    )

                # kv block (m=2) first so rope-k / v-transpose / attention can
                # begin while q projections still run.
                for m in (2, 0, 1):
                    for n in range(NJC):
                        emit_qkv_group(m, n)
                        if m in (0, 1):
                            emit_rope_q(m, n)
                    if m in (0, 1):
                        pass
                    else:
                        # rope k (partitions 0..63 of kv block), chunked
                        for n in range(NJC):
                            c0 = 2 * S + NQ * n
                            c1 = 2 * S + NQ * (n + 1)
                            swpk = ropetmp.tile(
                                [P, NQ], F32, tag="swp", name="swp", bufs=2
                            )
                            nc.vector.tensor_copy(
                                swpk[0:32, :], qkv_sb[32:64, c0:c1]
                            )
                            nc.vector.tensor_copy(
                                swpk[32:64, :], qkv_sb[0:32, c0:c1]
                            )
                            prodk = ropetmp.tile(
                                [P, NQ], F32, tag="prod", name="prod", bufs=2
                            )
                            nc.vector.tensor_mul(
                                prodk[0:64, :], qkv_sb[0:64, c0:c1],
                                ropec[0:64, NQ * n : NQ * (n + 1)],
                            )
                            prodk2 = ropetmp.tile(
                                [P, NQ], F32, tag="prod2", name="prod2", bufs=2
                            )
                            nc.vector.tensor_mul(
                                prodk2[0:64, :], swpk[0:64, :],
                                ropes[0:64, NQ * n : NQ * (n + 1)],
                            )
                            nc.vector.tensor_add(
                                kdup[0:64, NQ * n : NQ * (n + 1)],
                                prodk[0:64, :], prodk2[0:64, :],
                            )
                            nc.vector.tensor_add(
                                kdup[64:128, NQ * n : NQ * (n + 1)],
                                prodk[0:64, :], prodk2[0:64, :],
                            )
                        # transpose v -> v_aug [keys, 64 | 1]
                        for kb in range(NKB):
                            vps = ps_mm.tile([P, 64], F32, tag="mm")
                            nc.tensor.transpose(
                                vps[:, 0:64],
                                qkv_sb[64:128, 2 * S + P * kb : 2 * S + P * (kb + 1)],
                                ident[64:128, 64:128],
                            )
                            nc.vector.tensor_copy(
                                v_aug[:, kb * 65 : kb * 65 + 64], vps[:, 0:64]
                            )
                            nc.vector.memset(
                                v_aug[:, kb * 65 + 64 : kb * 65 + 65], 1.0
                            )

            # prefetch w_o tiles during attention (SBUF freed by ph1 close)
            wo_ctx = tc.tile_pool(name="wo_stream", bufs=4)
            wo_stream = wo_ctx.__enter__()
            wo_tiles = []
            for n in range(NJC):
                wos = wo_stream.tile([P, KT * NQ], BF16, tag="wos", name=f"wos{n}")
                nc.sync.dma_start(
                    wos[:].rearrange("p (kt c) -> p kt c", kt=KT),
                    wo_d[:].rearrange("p (kt c) -> p kt c", kt=KT)[
                        :, :, NQ * n : NQ * (n + 1)
                    ],
                )
                wo_tiles.append(wos)

            # ========== Phase 2: attention per head pair ==========
            cc_insts = []
            last_ccin = [None, None]
            with (
                tc.tile_pool(name="ps_s", bufs=2, space="PSUM") as ps_s,
                tc.tile_pool(name="ps_ctx", bufs=2, space="PSUM") as ps_ctx,
            ):
                for p in range(2):
                    for jc in range(NJC):
                        nkb = 4 * (jc + 1)
                        ctxs = [
                            ps_ctx.tile([P, NQ], F32, tag=f"ctx{hh}", name=f"ctx{hh}")
                            for hh in range(2)
                        ]
                        for kb in range(nkb):
                            d = kb - 4 * jc
                            sp = ps_s.tile([P, 2 * NQ], F32, tag="sp", name="sp")
                            for hh in range(2):
                                base = 64 * hh
                                nc.tensor.matmul(
                                    sp[:, NQ * hh : NQ * (hh + 1)],
                                    kdup[base : base + 64, P * kb : P * (kb + 1)],
                                    qpair[p][base : base + 64, NQ * jc : NQ * (jc + 1)],
                                    start=True,
                                    stop=True,
                                )
                            e = esb.tile([P, 2 * NQ], BF16, tag="e", name="e")
                            nc.scalar.activation(
                                e[:], sp[:], mybir.ActivationFunctionType.Exp,
                                scale=0.125,
                            )
                            if d >= 0:
                                nc.vector.tensor_mul(
                                    e[:], e[:],
                                    masks[:, 2 * NQ * d : 2 * NQ * (d + 1)],
                                )
                            for hh in range(2):
                                nc.tensor.matmul(
                                    ctxs[hh][0:65, :],
                                    v_aug[:, kb * 65 : kb * 65 + 65],
                                    e[:, NQ * hh : NQ * (hh + 1)],
                                    start=(kb == 0),
                                    stop=(kb == nkb - 1),
                                )
                        for hh in range(2):
                            h = 2 * p + hh
                            # reciprocal_approx_fast misreads PSUM at nonzero
                            # base partition -- stage the sum row through SBUF
                            rin = small.tile([1, NQ], F32, tag="rin")
                            nc.vector.tensor_copy(rin[:], ctxs[hh][64:65, :])
                            rec = small.tile([1, NQ], F32, tag="rec")
                            nc.vector.reciprocal_approx_fast(rec[:], rin[:])
                            bcs = small.tile([64, NQ], F32, tag="bcs")
                            nc.gpsimd.partition_broadcast(
                                bcs[:], rec[:], channels=64
                            )
                            ctxn = small.tile([64, NQ], BF16, tag="ctxn")
                            nc.vector.tensor_mul(ctxn[:], ctxs[hh][0:64, :], bcs[:])
                            # scatter: shard j (rows 128j..) holds my pair-p
                            # ctxT rows [64*hh ..] for core j's seq cols
                            for half in range(2):
                                j = 2 * jc + half
                                last_ccin[p] = nc.sync.dma_start(
                                    cc_in[p][
                                        P * j + 64 * hh : P * j + 64 * (hh + 1), :
                                    ],
                                    ctxn[:, SROWS * half : SROWS * (half + 1)],
                                )
                    # A2A for this pair; pair 0's collective overlaps pair 1's
                    # attention.
                    cc_insts.append(
                        nc.gpsimd.collective_compute(
                            "AllToAll",
                            mybir.AluOpType.bypass,
                            replica_groups=rg,
                            ins=[cc_in[p].opt()],
                            outs=[cc_out[p].opt()],
                        )
                    )

            # ========== Phase 3: o_proj on own seq rows ==========
            # qd chunk (2j + p) <- cc_out[p] rows [128j .. 128j+128)
            for j in range(NCORES):
                for p in range(2):
                    kt = 2 * j + p
                    dma = nc.sync.dma_start(
                        cc_sb[:, kt * SROWS : (kt + 1) * SROWS],
                        cc_out[p][j * P : (j + 1) * P, :],
                    )
                    # prevent Sync-queue head-of-line blocking: this DMA waits
                    # on collective completion, so don't let the scheduler
                    # hoist it ahead of pair-1's scatter traffic
                    _br.add_dep_helper(
                        dma.ins, last_ccin[1].ins, sync=True,
                        reason="cc_sb read after pair-1 scatter traffic",
                    )
            with (
                tc.tile_pool(name="ps_o", bufs=1, space="PSUM") as ps_o,
            ):
                # 8 PSUM banks, one per (n, m); even-kt chunks only need
                # cc_out[0] so this pass hides under the second AllToAll.
                o_ps = {}
                for n in range(NJC):
                    for m in range(2):
                        o_ps[(n, m)] = ps_o.tile(
                            [P, NQ], F32, tag=f"o{n}{m}", name=f"o{n}{m}"
                        )
                for parity in range(2):
                    for n in range(NJC):
                        for m in range(2):
                            for kk in range(KT // 2):
                                kt = 2 * kk + parity
                                nc.tensor.matmul(
                                    o_ps[(n, m)][:],
                                    cc_sb[
                                        :, kt * SROWS + P * m : kt * SROWS + P * (m + 1)
                                    ],
                                    wo_tiles[n][:, kt * NQ : (kt + 1) * NQ],
                                    start=(parity == 0 and kk == 0),
                                    stop=(parity == 1 and kk == KT // 2 - 1),
                                )
                            if parity == 1:
                                ot = outp.tile([P, NQ], F32, tag="ot")
                                nc.vector.tensor_copy(ot[:], o_ps[(n, m)][:])
                                nc.sync.dma_start(
                                    out_d[P * m : P * (m + 1), NQ * n : NQ * (n + 1)],
                                    ot[:],
                                )
            wo_ctx.__exit__(None, None, None)

    nc.compile()
    return nc


def _get_nc():
    global _NC_CACHE
    if _NC_CACHE is None:
        _NC_CACHE = _build_nc()
    return _NC_CACHE


def _stage_inputs(position_ids, hidden_states, w_qkv, w_o):
    """Host-side sharding / layout staging. Returns in_maps for 8 cores."""
    pos = np.asarray(position_ids)[0].astype(np.float32)            # [S]
    hidden = np.asarray(hidden_states, dtype=np.float32)[0]         # [S, H]
    w_qkv = np.asarray(w_qkv, dtype=np.float32)                     # [3072, H]
    w_o = np.asarray(w_o, dtype=np.float32)                         # [H, Q_SIZE]

    # hT tiles: [H, S] -> [128, KT*S] (k-tile kt at cols kt*S..)
    hT = np.ascontiguousarray(hidden.T)
    hT_r = np.ascontiguousarray(
        hT.reshape(KT, P, S).transpose(1, 0, 2).reshape(P, KT * S)
    ).astype(BF16_NP)

    # w_o.T tiles: [Q_SIZE, H] -> [128, KT*H]
    woT = np.ascontiguousarray(w_o.T)
    woT_r = np.ascontiguousarray(
        woT.reshape(KT, P, H).transpose(1, 0, 2).reshape(P, KT * H)
    ).astype(BF16_NP)

    # rope tables in [d, s] layout for a [128 = 2 heads x 64] tile
    inv_freq = (1.0 / (ROPE_THETA ** (np.arange(0, HD, 2, dtype=np.float32) / HD)))
    ang = pos[:, None] * inv_freq[None, :]                          # [S, 32]
    cosT = np.cos(ang).T.astype(np.float32)                         # [32, S]
    sinT = np.sin(ang).T.astype(np.float32)
    ropec = np.concatenate([cosT, cosT, cosT, cosT], axis=0)        # [128, S]
    ropes = np.concatenate([-sinT, sinT, -sinT, sinT], axis=0)      # [128, S]

    # causal masks for the 4 diagonal block offsets, stored duplicated
    # (legacy layout: [128, 8*NQ], block d at cols 2*NQ*d, first NQ used)
    f = np.arange(NQ)
    mask_list = []
    for d in range(4):
        pp = np.arange(P)[:, None] + 128 * d
        mk = (pp <= f[None, :]).astype(BF16_NP)
        mask_list.append(mk)
        mask_list.append(mk)
    masks = np.concatenate(mask_list, axis=1)                       # [128, 8*NQ]

    ident = np.eye(P, dtype=np.float32)

    in_maps = []
    for i in range(NCORES):
        rows_q = w_qkv[QH * HD * i : QH * HD * (i + 1)]             # [256, H]
        row_k = w_qkv[Q_SIZE + HD * i : Q_SIZE + HD * (i + 1)]      # [64, H]
        row_v = w_qkv[Q_SIZE + KV_SIZE + HD * i : Q_SIZE + KV_SIZE + HD * (i + 1)]
        wshard = np.concatenate([rows_q, row_k, row_v], axis=0)     # [384, H]
        wqkvT = np.ascontiguousarray(wshard.T)                      # [H, 384]
        wqkvT_r = np.ascontiguousarray(
            wqkvT.reshape(KT, P, 384).transpose(1, 0, 2).reshape(P, KT * 384)
        ).astype(BF16_NP)
        in_maps.append(
            {
                "hT": hT_r,
                "wqkvT": wqkvT_r,
                "woT": woT_r,
                "ropec": ropec,
                "ropes": ropes,
                "masks": masks,
                "ident": ident,
            }
        )
    return in_maps


def _ensure_ntff_hook():
    """The container's antenv stub lacks axon_hooks, so trn_boot silently
    skipped NTFF hook registration. Recreate the module and register the
    ctypes-based hook so run_bass_kernel_spmd(trace=True) can profile."""
    import sys
    import types

    if "antenv.axon_hooks" in sys.modules:
        return
    try:
        import antenv
        from trn_agent_boot.trn_boot import _ntff_profile_via_ctypes

        hooks = types.ModuleType("antenv.axon_hooks")
        _state = {}

        def set_axon_ntff_profile_hook(h):
            _state["h"] = h

        def get_axon_ntff_profile_hook():
            return _state.get("h")

        hooks.set_axon_ntff_profile_hook = set_axon_ntff_profile_hook
        hooks.get_axon_ntff_profile_hook = get_axon_ntff_profile_hook
        sys.modules["antenv.axon_hooks"] = hooks
        antenv.axon_hooks = hooks
        hook = _ntff_profile_via_ctypes("/opt/axon/libaxon_pjrt.so")
        if hook is not None:
            set_axon_ntff_profile_hook(hook)
    except Exception:
        pass


def kernel(**inputs):
    global LAST_RESULTS
    from concourse.bass_utils import run_bass_kernel_spmd

    nc = _get_nc()
    in_maps = _stage_inputs(
        inputs["position_ids"], inputs["hidden_states"], inputs["w_qkv"], inputs["w_o"]
    )
    trace = os.environ.get("KERNEL_TRACE", "0") == "1"
    if trace:
        _ensure_ntff_hook()
    res = run_bass_kernel_spmd(
        nc, in_maps, core_ids=list(range(NCORES)), trace=trace
    )
    LAST_RESULTS = res
    outs = [np.asarray(res.results[i]["out"], dtype=np.float32) for i in range(NCORES)]
    full = np.concatenate(outs, axis=0)                             # [S, H]
    return full.reshape(1, S, H)
